# revision 1
# baseline (speedup 1.0000x reference)
"""Trainium2 Bass kernel for nn_Caps2dMatwo (capsule conv + dual routing).

Strategy (8 NeuronCores, no cross-core communication):
  - Shard: core k handles batch n=k//4, H-rows [32*(k%4), 32*(k%4)+32).
  - PE stage: the 3x3 conv and the capsule pose/appearance transforms are
    fused into 9 per-tap matmuls over a permuted 128-channel axis
    (i,c,q,j), block-diagonal per 32-row i-block (4 row-tiled matmuls per
    tap).  The PSUM output holds, per pixel: u_hat (t,c,pa,i,k), the
    iteration-1 routing input p1 = 0.5*sum_c u_hat, and raw j=3 conv
    taps used for the coordinate-addition fixup.
  - Routing (3 iterations, sigmoid coupling, psquash/matwo_squash) runs
    on DVE/ACT/GPSIMD with pixels on partitions and capsule dims on the
    free axis (segmented free-axis reduces).
  - Output row is PE-transposed to channel-major and DMA'd contiguously.
"""
import os
from contextlib import ExitStack

import numpy as np

# problem constants (hardcoded per spec)
N, T0, T1 = 2, 4, 8
H, W = 128, 128
PD, AD = 4, 4
Z = 32
NBLK = 360          # psum cols per i-block: 256 own + 64 usum + 32 craw + 8 csum
ROWS = 32           # output rows per core
P = 128

_CACHE = {}


# ----------------------------------------------------------------- host prep
def _build_weights(W_conv, W_pos, W_app, b_app):
    """W_eff for the fused conv+pose matmul.

    Returns:
      w_in    [128, 9, 360]  per-partition weights, partition = i*32+c*8+q*4+j
      bias_uh [4c, 8t, 4k]   appearance bias term  b_app * colsum(Mapp)
      bias_p1 [8t, 4k]       0.5 * sum_c bias_uh
    """
    Kc = np.asarray(W_conv, np.float64)[:, :, :, 0, :]          # [c,dy,dx,t1]
    Mpos = np.asarray(W_pos, np.float64).reshape(T0, T1, PD, PD).copy()
    Mpos = Mpos / np.sqrt(np.maximum((Mpos ** 2).sum(axis=2, keepdims=True), 1e-12))
    Mapp = np.asarray(W_app, np.float64).reshape(T0, T1, AD, AD)
    Sapp = Mapp.sum(axis=2)                                      # [c,t,k]

    W_eff = np.zeros((9, 128, 4, NBLK), np.float64)
    for tap in range(9):
        dy, dx = tap // 3, tap % 3
        for i in range(4):
            for c in range(4):
                for q in range(2):
                    for j in range(4):
                        row = i * 32 + c * 8 + q * 4 + j
                        for t in range(q, 8, 2):
                            kpos = Kc[c, dy, dx, t // 2]
                            kapp = Kc[c, dy, dx, 4 + t // 2]
                            base = t * 32 + c * 8
                            W_eff[tap, row, i, base:base + 4] = kpos * Mpos[c, t, j]
                            W_eff[tap, row, i, base + 4:base + 8] = kapp * Mapp[c, t, j]
                            ub = 256 + t * 8
                            W_eff[tap, row, i, ub:ub + 4] += 0.5 * kpos * Mpos[c, t, j]
                            W_eff[tap, row, i, ub + 4:ub + 8] += 0.5 * kapp * Mapp[c, t, j]
                            if j == 3:
                                W_eff[tap, row, i, 320 + t * 4 + c] = kpos
                                W_eff[tap, row, i, 352 + t] += 0.5 * kpos
    # [9, 128, 4, 360] -> [128, 9, 360] picking each partition's own block
    w_in = np.zeros((128, 9, NBLK), np.float32)
    for i in range(4):
        w_in[i * 32:(i + 1) * 32] = W_eff[:, i * 32:(i + 1) * 32, i, :].transpose(1, 0, 2)
    bias_uh = np.einsum('ct,ctk->ctk', np.asarray(b_app, np.float64), Sapp)
    bias_p1 = 0.5 * bias_uh.sum(axis=0)
    return w_in, bias_uh.astype(np.float32), bias_p1.astype(np.float32)


def _shard_x(x):
    """x [N,T0,Z,H,W] -> list of 8 arrays [128, 34*130] (permuted channels)."""
    xp = np.zeros((N, T0, Z, H + 2, W + 2), np.float32)
    xp[:, :, :, 1:H + 1, 1:W + 1] = np.asarray(x, np.float32)
    # z = q*16 + i*4 + j ; partition = i*32 + c*8 + q*4 + j
    xq = xp.reshape(N, T0, 2, 4, 4, H + 2, W + 2)                # n c q i j h w
    xperm = np.ascontiguousarray(xq.transpose(0, 3, 1, 2, 4, 5, 6)
                                 ).reshape(N, 128, H + 2, W + 2)
    shards = []
    for core in range(8):
        n, rb = core // 4, (core % 4) * 32
        shards.append(np.ascontiguousarray(
            xperm[n, :, rb:rb + 34, :]).reshape(128, 34 * 130))
    return shards


# ------------------------------------------------------------- bass module
def _build_module():
    import concourse.bass as bass
    import concourse.tile as tile
    import concourse.mybir as mybir
    from concourse import bacc

    f32 = mybir.dt.float32
    f16 = mybir.dt.float16
    AX = mybir.AxisListType.X
    OP = mybir.AluOpType
    AF = mybir.ActivationFunctionType

    nc = bacc.Bacc("TRN2", num_devices=8, debug=False)
    x_d = nc.dram_tensor("x_shard", [128, 34 * 130], f16, kind="ExternalInput").ap()
    w_d = nc.dram_tensor("w_eff", [128, 9, NBLK], f16, kind="ExternalInput").ap()
    buh_d = nc.dram_tensor("bias_uh", [128, 512], f16, kind="ExternalInput").ap()
    bp1_d = nc.dram_tensor("bias_p1", [128, 128], f16, kind="ExternalInput").ap()
    cxy_d = nc.dram_tensor("cxy", [128, ROWS * 2], f32, kind="ExternalInput").ap()
    out_d = nc.dram_tensor("out_shard", [256, ROWS * 128], f16,
                           kind="ExternalOutput").ap()

    GRP = 8  # rows emitted per software-pipeline wave (batches ACT table sets)

    with tile.TileContext(nc) as tc, ExitStack() as ctx:
        const = ctx.enter_context(tc.tile_pool(name="const", bufs=1))
        work = ctx.enter_context(tc.tile_pool(name="work", bufs=GRP + 1))
        small = ctx.enter_context(tc.tile_pool(name="small", bufs=GRP + 1))
        psum = ctx.enter_context(tc.tile_pool(name="psum", bufs=2, space="PSUM"))

        x_sb = const.tile([P, 34, 130], f16)
        nc.sync.dma_start(out=x_sb[:].rearrange("p a b -> p (a b)"), in_=x_d)
        w_sb = const.tile([P, 9, NBLK], f16)
        nc.sync.dma_start(out=w_sb, in_=w_d)
        buh = const.tile([P, 8, 4, 16], f16)      # (t, c, (i k)) app bias, i-expanded
        nc.sync.dma_start(out=buh[:].rearrange("p a b c -> p (a b c)"), in_=buh_d)
        bp1 = const.tile([P, 8, 16], f16)         # (t, (i k)) p1 app bias, i-expanded
        nc.sync.dma_start(out=bp1[:].rearrange("p a b -> p (a b)"), in_=bp1_d)
        cxy = const.tile([P, ROWS, 2], f32)       # per row: (w/128, h/128)
        nc.sync.dma_start(out=cxy[:].rearrange("p a b -> p (a b)"), in_=cxy_d)
        eps_t = const.tile([P, 1], f32)
        nc.vector.memset(eps_t, 1e-9)
        tbuf = const.tile([P, 2, ROWS, 128], f16)  # (ch-half, r, w) output staging

        st = {}  # per-row live tiles

        def s0_matmul(r):
            ups = psum.tile([P, 2048], f32, tag="ups")
            st[r] = {"ups": ups}
            for tap in range(9):
                dy, dx = tap // 3, tap % 3
                patch = x_sb[:, r + dy, dx:dx + 128]
                for i in range(4):
                    nc.tensor.matmul(
                        ups[:, i * 512:i * 512 + 360],
                        lhsT=patch[32 * i:32 * (i + 1), :],
                        rhs=w_sb[32 * i:32 * (i + 1), tap, :],
                        start=(tap == 0), stop=(tap == 8),
                        tile_position=(32 * i, 0))

        def s1_assemble(r):
            ups = st[r]["ups"]
            uh = work.tile([P, 2, 8, 4, 16], f16, tag="uh")    # (pa, t, c, ik)
            p1 = work.tile([P, 2, 8, 16], f16, tag="p")        # (pa, t, ik)
            for i in range(4):
                own = ups[:, i * 512:i * 512 + 256].rearrange(
                    "p (t c pa k) -> p pa t c k", t=8, c=4, pa=2)
                nc.scalar.copy(uh[:, :, :, :, i * 4:(i + 1) * 4], own)
                usum = ups[:, i * 512 + 256:i * 512 + 320].rearrange(
                    "p (t pa k) -> p pa t k", t=8, pa=2)
                nc.scalar.copy(p1[:, :, :, i * 4:(i + 1) * 4], usum)
            upsb = ups[:].rearrange("p (i n) -> p i n", i=4)
            craw = small.tile([P, 8, 4, 4], f16, tag="craw")   # (t, c, i)
            nc.scalar.copy(craw, upsb[:, :, 320:352].rearrange(
                "p i (t c) -> p t c i", t=8))
            csum = small.tile([P, 8, 4], f16, tag="csum")      # (t, i)
            nc.scalar.copy(csum, upsb[:, :, 352:360].transpose([0, 2, 1]))

            nc.gpsimd.tensor_add(uh[:, 1], uh[:, 1], buh[:])
            nc.gpsimd.tensor_add(p1[:, 1], p1[:, 1], bp1[:])
            # coordinate addition: +cx*craw into k=0, +cy*craw into k=1
            tmp1 = small.tile([P, 2, 128], f16, tag="tmp1")
            tmp2 = small.tile([P, 2, 32], f16, tag="tmp2")
            for k in (0, 1):
                sc = cxy[:, r, k:k + 1]
                nc.vector.tensor_scalar_mul(
                    tmp1[:, k], craw[:].rearrange("p t c i -> p (t c i)"), sc)
                nc.vector.tensor_scalar_mul(
                    tmp2[:, k], csum[:].rearrange("p t i -> p (t i)"), sc)
            uv = uh[:, 0].rearrange("p t c (i k) -> p (t c) i k", i=4)[:, :, :, 0:2]
            nc.vector.tensor_add(
                uv, uv, tmp1[:].rearrange("p k (tc i) -> p tc i k", i=4))
            pv = p1[:, 0].rearrange("p t (i k) -> p t i k", i=4)[:, :, :, 0:2]
            nc.vector.tensor_add(
                pv, pv, tmp2[:].rearrange("p k (t i) -> p t i k", i=4))
            st[r].update(uh=uh, p=p1)

        def squash(r, vpos, vapp, vjoint=None):
            p = st[r]["p"]
            md = small.tile([P, 2, 8], f32, tag="md")   # [mx | den]
            nc.vector.tensor_reduce(out=md[:, 0], in_=p[:, 0], axis=AX,
                                    op=OP.max, apply_absolute_value=True)
            sq = small.tile([P, 8, 16], f32, tag="sq")
            nc.scalar.square(sq, p[:, 1])
            s = small.tile([P, 8], f32, tag="s")
            nc.vector.tensor_reduce(out=s, in_=sq, axis=AX, op=OP.add)
            sq1 = small.tile([P, 8], f32, tag="sq1")
            nc.scalar.activation(sq1, s, AF.Sqrt, bias=eps_t[:, 0:1])
            nc.vector.scalar_tensor_tensor(out=md[:, 1], in0=s, scalar=1.0,
                                           in1=sq1, op0=OP.add, op1=OP.mult)
            facs = small.tile([P, 2, 8], f16, tag="facs")   # [rmx | gf]
            with nc.allow_low_precision("v factors consumed in fp16 anyway"):
                nc.vector.reciprocal(facs[:], md[:])
            nc.vector.tensor_mul(facs[:, 1], s, facs[:, 1])
            if vjoint is not None:
                nc.vector.tensor_mul(
                    vjoint, p[:], facs[:].unsqueeze(3).broadcast_to((P, 2, 8, 16)))
            else:
                nc.vector.tensor_mul(vpos, p[:, 0],
                                     facs[:, 0].unsqueeze(2).broadcast_to((P, 8, 16)))
                nc.vector.tensor_mul(vapp, p[:, 1],
                                     facs[:, 1].unsqueeze(2).broadcast_to((P, 8, 16)))

        def rout_badd(r, first):
            uh, v, b2 = st[r]["uh"], st[r]["v"], st[r].get("b2")
            uhm = uh[:].rearrange("p pa t c ik -> p (pa t) c ik")
            vm = (v[:].rearrange("p pa t ik -> p (pa t) ik")
                  .unsqueeze(2).broadcast_to((P, 16, 4, 16)))
            wp = work.tile([P, 16, 4, 16], f16, tag="wp")
            nc.vector.tensor_mul(wp, uhm, vm)
            wa = work.tile([P, 16, 4, 8], f16, tag="wa")
            nc.vector.tensor_add(wa, wp[:, :, :, 0:8], wp[:, :, :, 8:16])
            wb = work.tile([P, 16, 4, 4], f16, tag="wb")
            nc.vector.tensor_add(wb, wa[:, :, :, 0:4], wa[:, :, :, 4:8])
            wc = work.tile([P, 16, 4, 2], f16, tag="wc")
            nc.vector.tensor_add(wc, wb[:, :, :, 0:2], wb[:, :, :, 2:4])
            ab = small.tile([P, 2, 8, 4], f16, tag="ab")
            nc.vector.tensor_add(ab[:].rearrange("p pa t c -> p (pa t) c"),
                                 wc[:, :, :, 0], wc[:, :, :, 1])
            if first:
                b2 = work.tile([P, 2, 8, 4], f32, tag="b")
                st[r]["b2"] = b2
                nc.vector.tensor_mul(b2[:, 0], ab[:, 0], ab[:, 1])
                nc.vector.tensor_mul(b2[:, 1], ab[:, 0], ab[:, 1])
            else:
                rt = small.tile([P, 8, 4], f32, tag="rt")
                nc.vector.tensor_mul(rt, ab[:, 0], ab[:, 1])
                nc.vector.tensor_add(b2[:, 0], b2[:, 0], rt)
                nc.vector.tensor_add(b2[:, 1], b2[:, 1], rt)

        def sig_p(r, gp):
            uh, b2 = st[r]["uh"], st[r]["b2"]
            r2 = work.tile([P, 2, 8, 4], f16, tag="rt_sig")
            nc.scalar.activation(r2, b2, AF.Sigmoid)
            uhm = uh[:].rearrange("p pa t c ik -> p (pa t) c ik")
            rv = (r2[:].rearrange("p pa t c -> p (pa t) c")
                  .unsqueeze(3).broadcast_to((P, 16, 4, 16)))
            m = work.tile([P, 2, 8, 4, 16], f16, tag="m")
            p = work.tile([P, 2, 8, 16], f16, tag="p")
            eng = nc.gpsimd if gp else nc.vector
            eng.tensor_mul(
                m[:].rearrange("p pa t c ik -> p (pa t) c ik"), uhm, rv)
            ta = work.tile([P, 2, 8, 16], f16, tag="ta")
            eng.tensor_add(ta[:], m[:, :, :, 0, :], m[:, :, :, 1, :])
            tb = work.tile([P, 2, 8, 16], f16, tag="tb")
            eng.tensor_add(tb[:], m[:, :, :, 2, :], m[:, :, :, 3, :])
            eng.tensor_add(p[:], ta[:], tb[:])
            st[r]["p"] = p

        def s2_squash1_rout(r):
            v = work.tile([P, 2, 8, 16], f16, tag="v")
            st[r]["v"] = v
            squash(r, None, None, vjoint=v[:])
            rout_badd(r, first=True)

        def s3_sig_p2(r):
            sig_p(r, gp=False)

        def s4_squash2_rout(r):
            v = work.tile([P, 2, 8, 16], f16, tag="v")
            st[r]["v"] = v
            squash(r, None, None, vjoint=v[:])
            rout_badd(r, first=False)

        def s5_sig_p3(r):
            sig_p(r, gp=True)

        def s6_squash3_out(r):
            v3 = work.tile([P, 8, 2, 16], f16, tag="v3")
            squash(r, v3[:, :, 0, :], v3[:, :, 1, :])
            vflat = v3[:].rearrange("p t pa ik -> p (t pa ik)")
            for half in (0, 1):
                nc.sync.dma_start_transpose(
                    tbuf[:, half, r, :], vflat[:, half * 128:(half + 1) * 128])
            del st[r]

        stages = [s0_matmul, s1_assemble, s2_squash1_rout, s3_sig_p2,
                  s4_squash2_rout, s5_sig_p3, s6_squash3_out]
        for g in range(0, ROWS, GRP):
            rows = range(g, min(g + GRP, ROWS))
            for stage in stages:
                for r in rows:
                    stage(r)

        nc.sync.dma_start(out=out_d[0:128, :],
                          in_=tbuf[:, 0].rearrange("p a b -> p (a b)"))
        nc.sync.dma_start(out=out_d[128:256, :],
                          in_=tbuf[:, 1].rearrange("p a b -> p (a b)"))

    nc.compile()
    return nc


def _make_in_map(core, shards, w_in, bias_uh, bias_p1):
    """Per-core input dict. bias_uh [c,t,k] and bias_p1 [t,k] get i-expanded."""
    rb = (core % 4) * 32
    # (t, c, (i, k)) with i broadcast
    buh_in = np.broadcast_to(
        bias_uh.transpose(1, 0, 2)[:, :, None, :], (8, 4, 4, 4)).reshape(1, 512)
    buh_in = np.broadcast_to(buh_in, (128, 512)).copy()
    bp1_in = np.broadcast_to(
        bias_p1[:, None, :], (8, 4, 4)).reshape(1, 128)
    bp1_in = np.broadcast_to(bp1_in, (128, 128)).copy()
    cxy_in = np.zeros((128, ROWS, 2), np.float32)
    cxy_in[:, :, 0] = (np.arange(128, dtype=np.float32) / 128.0)[:, None]
    cxy_in[:, :, 1] = ((rb + np.arange(ROWS, dtype=np.float32)) / 128.0)[None, :]
    return {
        "x_shard": shards[core].astype(np.float16),
        "w_eff": w_in.astype(np.float16),
        "bias_uh": buh_in.astype(np.float16),
        "bias_p1": bp1_in.astype(np.float16),
        "cxy": cxy_in.reshape(128, ROWS * 2),
    }


def kernel(x, W_conv, W_pos, W_app, b_app):
    from concourse.bass_utils import run_bass_kernel_spmd

    if "nc" not in _CACHE:
        _CACHE["nc"] = _build_module()
    nc = _CACHE["nc"]

    w_in, bias_uh, bias_p1 = _build_weights(W_conv, W_pos, W_app, b_app)
    shards = _shard_x(x)
    in_maps = [_make_in_map(core, shards, w_in, bias_uh, bias_p1)
               for core in range(8)]

    trace = bool(int(os.environ.get("CAPS_TRACE", "0")))
    res = run_bass_kernel_spmd(nc, in_maps, core_ids=list(range(8)), trace=trace)
    _CACHE["last_result"] = res

    out = np.zeros((N, T1, Z, H, W), np.float32)
    for core in range(8):
        n, rb = core // 4, (core % 4) * 32
        o = res.results[core]["out_shard"].astype(np.float32).reshape(
            8, 32, ROWS, 128)
        out[n, :, :, rb:rb + 32, :] = o
    return out



# revision 15
# speedup vs baseline: 1.3577x; 1.3577x over previous
"""Trainium2 Bass kernel for nn_Caps2dMatwo (capsule conv + dual routing).

Strategy (8 NeuronCores, no cross-core communication):
  - Shard: core k handles batch n=k//4, H-rows [32*(k%4), 32*(k%4)+32).
  - PE stage: 3x3 conv + capsule pose/appearance transforms fused into 9
    per-tap matmuls over a permuted 128-channel axis (i,c,q,j),
    block-diagonal per 32-row i-block.  PSUM per i-block emits:
      [0:256)   u_hat own cols, order (pa, c, t, k)
      [256:320) usum:  0.5*sum_c u_hat  (pa, t, k)  -> p1
      [320:384) craw2: raw j=3 conv taps (c, t, k2) duplicated x2
      [384:400) csum2: 0.5*sum_c raw j=3  (t, k2)   duplicated x2
  - Routing (3 iterations) on DVE/ACT/POOL, pixels on partitions,
    batched 8 rows per instruction.  uh layout (pa, c, t, i, k): every
    broadcast is an outer dim over a contiguous block, so all fat
    elementwise ops are single 3D-AP instructions in DVE 2x mode.
    Sigmoid on ACT writes its output pre-expanded over (i,k);
    sqrt/reciprocal use DVE integer-hack Newton iterations so only the
    sigmoid ACT table set is ever loaded.
  - Output row is DMA-transposed to channel-major and written out.
"""
import os
from contextlib import ExitStack

import numpy as np

# problem constants (hardcoded per spec)
N, T0, T1 = 2, 4, 8
H, W = 128, 128
PD, AD = 4, 4
Z = 32
NBLK = 400          # psum cols per i-block: 256 own + 64 usum + 64 craw2 + 16 csum2
ROWS = 32           # output rows per core
P = 128
R = 8               # rows per routing group
NG = ROWS // R

_CACHE = {}


# ----------------------------------------------------------------- host prep
def _build_weights(W_conv, W_pos, W_app, b_app):
    """W_eff for the fused conv+pose matmul.

    Returns:
      w_in  [128, 9, 400]  per-partition weights, partition = i*32+c*8+q*4+j
      buh   [128, 512]     app bias b_app*colsum(Mapp), layout (c,t,i,k)
      bp1   [128, 128]     p1 app bias 0.5*sum_c buh, layout (t,i,k)
    """
    Kc = np.asarray(W_conv, np.float64)[:, :, :, 0, :]          # [c,dy,dx,t1]
    Mpos = np.asarray(W_pos, np.float64).reshape(T0, T1, PD, PD).copy()
    Mpos = Mpos / np.sqrt(np.maximum((Mpos ** 2).sum(axis=2, keepdims=True), 1e-12))
    Mapp = np.asarray(W_app, np.float64).reshape(T0, T1, AD, AD)
    Sapp = Mapp.sum(axis=2)                                      # [c,t,k]

    W_eff = np.zeros((9, 128, 4, NBLK), np.float64)
    for tap in range(9):
        dy, dx = tap // 3, tap % 3
        for i in range(4):
            for c in range(4):
                for q in range(2):
                    for j in range(4):
                        row = i * 32 + c * 8 + q * 4 + j
                        for t in range(q, 8, 2):
                            kpos = Kc[c, dy, dx, t // 2]
                            kapp = Kc[c, dy, dx, 4 + t // 2]
                            # own cols (pa, c, t, k)
                            b0 = 0 * 128 + c * 32 + t * 4
                            W_eff[tap, row, i, b0:b0 + 4] = kpos * Mpos[c, t, j]
                            b1 = 1 * 128 + c * 32 + t * 4
                            W_eff[tap, row, i, b1:b1 + 4] = kapp * Mapp[c, t, j]
                            # usum (pa, t, k) with 0.5
                            u0 = 256 + 0 * 32 + t * 4
                            W_eff[tap, row, i, u0:u0 + 4] += 0.5 * kpos * Mpos[c, t, j]
                            u1 = 256 + 1 * 32 + t * 4
                            W_eff[tap, row, i, u1:u1 + 4] += 0.5 * kapp * Mapp[c, t, j]
                            if j == 3:
                                # craw2 (c, t, k2) duplicated
                                cr = 320 + c * 16 + t * 2
                                W_eff[tap, row, i, cr:cr + 2] = kpos
                                # csum2 (t, k2) with 0.5, accumulated over c
                                cs = 384 + t * 2
                                W_eff[tap, row, i, cs:cs + 2] += 0.5 * kpos
    # [9, 128, 4, 400] -> [128, 9, 400] picking each partition's own block
    w_in = np.zeros((128, 9, NBLK), np.float32)
    for i in range(4):
        w_in[i * 32:(i + 1) * 32] = W_eff[:, i * 32:(i + 1) * 32, i, :].transpose(1, 0, 2)
    buh = np.einsum('ct,ctk->ctk', np.asarray(b_app, np.float64), Sapp)  # (c,t,k)
    # i-expanded host-side: (c, t, i, k) and (t, i, k)
    buh_e = np.broadcast_to(buh[:, :, None, :], (4, 8, 4, 4)).reshape(1, 512)
    bp1 = 0.5 * buh.sum(axis=0)                                  # (t,k)
    bp1_e = np.broadcast_to(bp1[:, None, :], (8, 4, 4)).reshape(1, 128)
    buh_in = np.broadcast_to(buh_e, (128, 512)).copy()
    bp1_in = np.broadcast_to(bp1_e, (128, 128)).copy()
    return w_in, buh_in.astype(np.float32), bp1_in.astype(np.float32)


def _shard_x(x):
    """x [N,T0,Z,H,W] -> list of 8 arrays [128, 34*130] (permuted channels)."""
    xp = np.zeros((N, T0, Z, H + 2, W + 2), np.float32)
    xp[:, :, :, 1:H + 1, 1:W + 1] = np.asarray(x, np.float32)
    # z = q*16 + i*4 + j ; partition = i*32 + c*8 + q*4 + j
    xq = xp.reshape(N, T0, 2, 4, 4, H + 2, W + 2)                # n c q i j h w
    xperm = np.ascontiguousarray(xq.transpose(0, 3, 1, 2, 4, 5, 6)
                                 ).reshape(N, 128, H + 2, W + 2)
    shards = []
    for core in range(8):
        n, rb = core // 4, (core % 4) * 32
        shards.append(np.ascontiguousarray(
            xperm[n, :, rb:rb + 34, :]).reshape(128, 34 * 130))
    return shards


# ------------------------------------------------------------- bass module
def _build_module():
    import concourse.bass as bass
    import concourse.tile as tile
    import concourse.mybir as mybir
    from concourse import bacc

    f32 = mybir.dt.float32
    f16 = mybir.dt.float16
    i32 = mybir.dt.int32
    AX = mybir.AxisListType
    OP = mybir.AluOpType
    AF = mybir.ActivationFunctionType

    MAGIC = 0x5F3759DF  # fp32 rsqrt seed

    nc = bacc.Bacc("TRN2", num_devices=8, debug=False)
    x_d = nc.dram_tensor("x_shard", [128, 34 * 130], f16, kind="ExternalInput").ap()
    w_d = nc.dram_tensor("w_eff", [128, 9, NBLK], f16, kind="ExternalInput").ap()
    buh_d = nc.dram_tensor("bias_uh", [128, 512], f16, kind="ExternalInput").ap()
    bp1_d = nc.dram_tensor("bias_p1", [128, 128], f16, kind="ExternalInput").ap()
    cxy_d = nc.dram_tensor("cxy", [128, ROWS * 2], f16, kind="ExternalInput").ap()
    out_d = nc.dram_tensor("out_shard", [256, ROWS * 128], f16,
                           kind="ExternalOutput").ap()

    with tile.TileContext(nc) as tc, ExitStack() as ctx:
        const = ctx.enter_context(tc.tile_pool(name="const", bufs=1))
        grp = ctx.enter_context(tc.tile_pool(name="grp", bufs=2))
        scr = ctx.enter_context(tc.tile_pool(name="scr", bufs=1))
        sml = ctx.enter_context(tc.tile_pool(name="sml", bufs=2))
        psum = ctx.enter_context(tc.tile_pool(name="psum", bufs=2, space="PSUM"))

        x_sb = const.tile([P, 34, 130], f16)
        nc.sync.dma_start(out=x_sb[:].rearrange("p a b -> p (a b)"), in_=x_d)
        w_sb = const.tile([P, 9, NBLK], f16)
        nc.sync.dma_start(out=w_sb, in_=w_d)
        buh = const.tile([P, 512], f16)           # (c, t, i, k) app bias
        nc.sync.dma_start(out=buh, in_=buh_d)
        bp1 = const.tile([P, 128], f16)           # (t, i, k) p1 app bias
        nc.sync.dma_start(out=bp1, in_=bp1_d)
        cxy = const.tile([P, ROWS, 2], f16)       # per row: (w/128, h/128)
        nc.sync.dma_start(out=cxy[:].rearrange("p a b -> p (a b)"), in_=cxy_d)
        ones_i = const.tile([P, R, 8], i32)
        nc.vector.memset(ones_i, 1)
        magic_i = const.tile([P, R, 8], i32)
        nc.vector.memset(magic_i, MAGIC)
        tbuf = const.tile([P, 2, ROWS, 128], f16)  # (ch-half, r, w) output staging

        # ---------------- per-row: matmul + assemble into group tiles
        def s0_matmul(r):
            ups = psum.tile([P, 2048], f32, tag="ups")
            for tap in range(9):
                dy, dx = tap // 3, tap % 3
                patch = x_sb[:, r + dy, dx:dx + 128]
                for i in range(4):
                    nc.tensor.matmul(
                        ups[:, i * 512:i * 512 + NBLK],
                        lhsT=patch[32 * i:32 * (i + 1), :],
                        rhs=w_sb[32 * i:32 * (i + 1), tap, :],
                        start=(tap == 0), stop=(tap == 8),
                        tile_position=(32 * i, 0))
            return ups

        def s1_assemble(g, r, ups, uh, p1, cr2, cs2):
            """Copy psum -> group tiles (ACT engine only; Copy needs no table)."""
            upsb = ups[:].rearrange("p (i n) -> p i n", i=4)
            # uh (pa, c, t, i, k): src block cols pa*128 + c*32 + t*4 + k
            for pa in range(2):
                src = upsb[:, :, pa * 128:(pa + 1) * 128].rearrange(
                    "p i (c t k) -> p c t i k", c=4, t=8)
                nc.scalar.copy(uh[:, r, pa], src)
            # p1 (pa, t, i, k): src usum cols 256 + pa*32 + t*4 + k
            nc.scalar.copy(p1[:, r], upsb[:, :, 256:320].rearrange(
                "p i (pa t k) -> p pa t i k", pa=2, t=8))
            # craw2 (c, t, i, k2): src cols 320 + c*16 + t*2 + k2
            nc.scalar.copy(cr2[:, r], upsb[:, :, 320:384].rearrange(
                "p i (c t k) -> p c t i k", c=4, t=8))
            # csum2 (t, i, k2): src cols 384 + t*2 + k2
            nc.scalar.copy(cs2[:, r], upsb[:, :, 384:400].rearrange(
                "p i (t k) -> p t i k", t=8))

        def s2_bias_coord(g, uh, p1, cr2, cs2):
            """App biases + coordinate addition, group-batched."""
            # uh app half += buh (one POOL op; (c,t,i,k) contiguous both sides)
            uha = uh[:, :, 1].rearrange("p r c t i k -> p r (c t i k)")
            nc.gpsimd.tensor_add(
                uha, uha, buh[:].unsqueeze(1).broadcast_to((P, R, 512)))
            # p1 app half += bp1
            p1a = p1[:, :, 1].rearrange("p r t i k -> p r (t i k)")
            nc.vector.tensor_add(
                p1a, p1a, bp1[:].unsqueeze(1).broadcast_to((P, R, 128)))
            # coordinate addition: uh[pos, c, t, i, k0:2] += cxy * craw2
            cxs = cxy[:, g * R:(g + 1) * R, :]      # [P, R, 2]
            tmp = scr.tile([P, R, 128, 2], f16, tag="ctmp")  # ((c t i), k2)
            nc.vector.tensor_mul(
                tmp, cr2[:].rearrange("p r c t i k -> p r (c t i) k"),
                cxs.unsqueeze(2).broadcast_to((P, R, 128, 2)))
            uv = uh[:, :, 0, :, :, :, 0:2]
            nc.vector.tensor_add(uv, uv, tmp[:].rearrange(
                "p r (c t i) k -> p r c t i k", c=4, t=8))
            tmp2 = scr.tile([P, R, 32, 2], f16, tag="cstmp")  # ((t i), k2)
            nc.vector.tensor_mul(
                tmp2, cs2[:].rearrange("p r t i k -> p r (t i) k"),
                cxs.unsqueeze(2).broadcast_to((P, R, 32, 2)))
            pv = p1[:, :, 0, :, :, 0:2]
            nc.vector.tensor_add(pv, pv, tmp2[:].rearrange(
                "p r (t i) k -> p r t i k", t=8))

        # ---------------- group-batched routing helpers
        def squash_factors(g, p, rmx, gf):
            """rmx = 1/max|p_pos|, gf = s/((1+s)sqrt(s+eps)); all [P,R,8] f32."""
            # pos: max|p| via tensor_reduce (abs+max), then fast reciprocal
            mx = sml.tile([P, R, 8], f32, tag="mx")
            nc.vector.tensor_reduce(
                out=mx, in_=p[:, :, 0].rearrange("p r t i k -> p r t (i k)"),
                axis=AX.X, op=OP.max, apply_absolute_value=True)
            nc.vector.reciprocal_approx_fast(
                out=rmx[:].rearrange("p r t -> p (r t)"),
                in_=mx[:].rearrange("p r t -> p (r t)"))
            # app: s = sum p^2 (POOL mul/tree + DVE tail), layout (t, i, k)
            sq = scr.tile([P, R, 8, 4, 4], f16, tag="sq")
            nc.gpsimd.tensor_mul(sq, p[:, :, 1], p[:, :, 1])
            t8 = scr.tile([P, R, 8, 2, 4], f16, tag="sqt8")
            nc.gpsimd.tensor_add(t8, sq[:, :, :, 0:2, :], sq[:, :, :, 2:4, :])
            t4 = sml.tile([P, R, 8, 4], f16, tag="sqt4")
            nc.vector.tensor_add(t4, t8[:, :, :, 0], t8[:, :, :, 1])
            t2 = sml.tile([P, R, 8, 2], f16, tag="sqt2")
            nc.vector.tensor_add(t2, t4[:, :, :, 0:2], t4[:, :, :, 2:4])
            s = sml.tile([P, R, 8], f32, tag="s")
            nc.vector.tensor_add(s, t2[:, :, :, 0], t2[:, :, :, 1])
            # rs = rsqrt(s + eps) via int hack + 2 Newton iterations
            x = sml.tile([P, R, 8], f32, tag="x")
            nc.vector.tensor_scalar_add(x, s, 1e-9)
            xi = x[:].bitcast(i32)
            y = sml.tile([P, R, 8], f32, tag="y")
            yi = y[:].bitcast(i32)
            nc.vector.tensor_tensor(out=yi, in0=xi, in1=ones_i[:],
                                    op=OP.logical_shift_right)
            nc.vector.tensor_tensor(out=yi, in0=magic_i[:], in1=yi,
                                    op=OP.subtract)
            for _ in range(2):
                y2 = sml.tile([P, R, 8], f32, tag="y2")
                nc.vector.tensor_mul(y2, y, y)
                nc.vector.tensor_mul(y2, y2, x)
                nc.vector.tensor_scalar(out=y2, in0=y2, scalar1=-0.5,
                                        scalar2=1.5, op0=OP.mult, op1=OP.add)
                nc.vector.tensor_mul(y, y, y2)
            # r1s = 1/(1+s); gf = s * rs * r1s
            o1 = sml.tile([P, R, 8], f32, tag="o1")
            nc.vector.tensor_scalar_add(o1, s, 1.0)
            r1s = sml.tile([P, R, 8], f32, tag="r1s")
            nc.vector.reciprocal_approx_fast(
                out=r1s[:].rearrange("p r t -> p (r t)"),
                in_=o1[:].rearrange("p r t -> p (r t)"))
            nc.vector.tensor_mul(gf, s, y)
            nc.vector.tensor_mul(gf, gf, r1s)

        def rout(g, uh, p, rmx, gf, b, first):
            """b += (sum_ik uh_pos*p_pos)*(sum_ik uh_app*p_app)*rmx*gf.

            b layout [P, R, 4, 8] = (c, t), fp32."""
            wp = scr.tile([P, R, 2, 4, 8, 16], f16, tag="wp")  # (pa,c,t,ik)
            for pa in range(2):
                nc.vector.tensor_mul(
                    wp[:, :, pa].rearrange("p r c t ik -> p r c (t ik)"),
                    uh[:, :, pa].rearrange("p r c t i k -> p r c (t i k)"),
                    p[:, :, pa].rearrange("p r t i k -> p r (t i k)")
                    .unsqueeze(2).broadcast_to((P, R, 4, 128)))
            # reduce over ik (keep pa,c,t)
            wpv = wp[:].rearrange("p r pa c t ik -> p r (pa c t) ik")
            wa = scr.tile([P, R, 64, 8], f16, tag="wa")
            nc.vector.tensor_add(wa, wpv[:, :, :, 0:8], wpv[:, :, :, 8:16])
            wb = scr.tile([P, R, 64, 4], f16, tag="wb")
            nc.vector.tensor_add(wb, wa[:, :, :, 0:4], wa[:, :, :, 4:8])
            wc = scr.tile([P, R, 64, 2], f16, tag="wc")
            nc.vector.tensor_add(wc, wb[:, :, :, 0:2], wb[:, :, :, 2:4])
            ab = scr.tile([P, R, 64], f16, tag="ab")   # (pa, c, t)
            nc.vector.tensor_add(ab, wc[:, :, :, 0], wc[:, :, :, 1])
            # rout = ab_pos * ab_app * (rmx*gf)
            u1 = sml.tile([P, R, 4, 8], f32, tag="u1")
            nc.vector.tensor_mul(u1[:].rearrange("p r c t -> p r (c t)"),
                                 ab[:, :, 0:32], ab[:, :, 32:64])
            fn = sml.tile([P, R, 8], f32, tag="fn")
            nc.vector.tensor_mul(fn, rmx, gf)
            fnb = fn[:].unsqueeze(2).broadcast_to((P, R, 4, 8))
            if first:
                nc.vector.tensor_mul(b, u1, fnb)
            else:
                rt = sml.tile([P, R, 4, 8], f32, tag="rt")
                nc.vector.tensor_mul(rt, u1, fnb)
                nc.vector.tensor_add(b, b, rt)

        def sig_p(g, uh, b, p):
            """rE = sigmoid(b) expanded over (i,k) on ACT; p = sum_c uh*rE."""
            rE = scr.tile([P, R, 4, 8, 16], f16, tag="rE")  # (c, t, ik)
            nc.scalar.activation(
                rE[:].rearrange("p r c t ik -> p r (c t) ik"),
                b[:].rearrange("p r c t -> p r (c t)")
                .unsqueeze(3).broadcast_to((P, R, 32, 16)),
                AF.Sigmoid)
            m = scr.tile([P, R, 2, 4, 8, 16], f16, tag="m")  # (pa,c,t,ik)
            for pa in range(2):
                nc.vector.tensor_mul(
                    m[:, :, pa].rearrange("p r c t ik -> p r (c t ik)"),
                    uh[:, :, pa].rearrange("p r c t i k -> p r (c t i k)"),
                    rE[:].rearrange("p r c t ik -> p r (c t ik)"))
            # sum over c: (pa, c, t, ik) -> (pa, t, ik)
            ta = scr.tile([P, R, 2, 2, 8, 16], f16, tag="ta")
            nc.vector.tensor_add(ta, m[:, :, :, 0:2], m[:, :, :, 2:4])
            nc.vector.tensor_add(
                p[:].rearrange("p r pa t i k -> p r pa (t i k)"),
                ta[:, :, :, 0].rearrange("p r pa t ik -> p r pa (t ik)"),
                ta[:, :, :, 1].rearrange("p r pa t ik -> p r pa (t ik)"))

        def s_out(g, p, rmx, gf):
            """v3 = [p_pos*rmx | p_app*gf] -> (t, pa, i, k); DMA transpose out."""
            fac2 = sml.tile([P, R, 8, 2], f32, tag="fac2")  # (t, pa)
            nc.vector.tensor_copy(fac2[:, :, :, 0], rmx)
            nc.vector.tensor_copy(fac2[:, :, :, 1], gf)
            facE = scr.tile([P, R, 8, 2, 16], f16, tag="facE")  # (t, pa, ik)
            nc.scalar.activation(
                facE[:].rearrange("p r t pa ik -> p r (t pa) ik"),
                fac2[:].rearrange("p r t pa -> p r (t pa)")
                .unsqueeze(3).broadcast_to((P, R, 16, 16)),
                AF.Copy)
            v3 = scr.tile([P, R, 8, 2, 16], f16, tag="v3")  # (t, pa, ik)
            for r in range(R):
                nc.vector.tensor_mul(
                    v3[:, r],
                    p[:, r].rearrange("p pa t i k -> p pa t (i k)")
                    .transpose([0, 2, 1, 3]),
                    facE[:, r])
            vflat = v3[:].rearrange("p r t pa ik -> p r (t pa ik)")
            for r in range(R):
                row = g * R + r
                for half in (0, 1):
                    nc.sync.dma_start_transpose(
                        tbuf[:, half, row, :],
                        vflat[:, r, half * 128:(half + 1) * 128])

        # ---------------- main schedule
        for g in range(NG):
            uh = grp.tile([P, R, 2, 4, 8, 4, 4], f16, tag="uh")   # (pa,c,t,i,k)
            p1 = grp.tile([P, R, 2, 8, 4, 4], f16, tag="p1")      # (pa,t,i,k)
            cr2 = grp.tile([P, R, 4, 8, 4, 2], f16, tag="cr2")    # (c,t,i,k2)
            cs2 = grp.tile([P, R, 8, 4, 2], f16, tag="cs2")       # (t,i,k2)
            for r in range(R):
                ups = s0_matmul(g * R + r)
                s1_assemble(g, r, ups, uh, p1, cr2, cs2)
            s2_bias_coord(g, uh, p1, cr2, cs2)

            rmx1 = sml.tile([P, R, 8], f32, tag="rmx1")
            gf1 = sml.tile([P, R, 8], f32, tag="gf1")
            squash_factors(g, p1, rmx1, gf1)
            b = grp.tile([P, R, 4, 8], f32, tag="b")
            rout(g, uh, p1, rmx1, gf1, b, first=True)

            p2 = grp.tile([P, R, 2, 8, 4, 4], f16, tag="p2")
            sig_p(g, uh, b, p2)
            rmx2 = sml.tile([P, R, 8], f32, tag="rmx2")
            gf2 = sml.tile([P, R, 8], f32, tag="gf2")
            squash_factors(g, p2, rmx2, gf2)
            rout(g, uh, p2, rmx2, gf2, b, first=False)

            p3 = grp.tile([P, R, 2, 8, 4, 4], f16, tag="p3")
            sig_p(g, uh, b, p3)
            rmx3 = sml.tile([P, R, 8], f32, tag="rmx3")
            gf3 = sml.tile([P, R, 8], f32, tag="gf3")
            squash_factors(g, p3, rmx3, gf3)
            s_out(g, p3, rmx3, gf3)

        nc.sync.dma_start(out=out_d[0:128, :],
                          in_=tbuf[:, 0].rearrange("p a b -> p (a b)"))
        nc.sync.dma_start(out=out_d[128:256, :],
                          in_=tbuf[:, 1].rearrange("p a b -> p (a b)"))

    nc.compile()
    return nc


def _make_in_map(core, shards, w_in, buh, bp1):
    rb = (core % 4) * 32
    cxy_in = np.zeros((128, ROWS, 2), np.float32)
    cxy_in[:, :, 0] = (np.arange(128, dtype=np.float32) / 128.0)[:, None]
    cxy_in[:, :, 1] = ((rb + np.arange(ROWS, dtype=np.float32)) / 128.0)[None, :]
    return {
        "x_shard": shards[core].astype(np.float16),
        "w_eff": w_in.astype(np.float16),
        "bias_uh": buh.astype(np.float16),
        "bias_p1": bp1.astype(np.float16),
        "cxy": cxy_in.reshape(128, ROWS * 2).astype(np.float16),
    }


def kernel(x, W_conv, W_pos, W_app, b_app):
    from concourse.bass_utils import run_bass_kernel_spmd

    if "nc" not in _CACHE:
        _CACHE["nc"] = _build_module()
    nc = _CACHE["nc"]

    w_in, buh, bp1 = _build_weights(W_conv, W_pos, W_app, b_app)
    shards = _shard_x(x)
    in_maps = [_make_in_map(core, shards, w_in, buh, bp1)
               for core in range(8)]

    trace = bool(int(os.environ.get("CAPS_TRACE", "0")))
    res = run_bass_kernel_spmd(nc, in_maps, core_ids=list(range(8)), trace=trace)
    _CACHE["last_result"] = res

    out = np.zeros((N, T1, Z, H, W), np.float32)
    for core in range(8):
        n, rb = core // 4, (core % 4) * 32
        o = res.results[core]["out_shard"].astype(np.float32).reshape(
            8, 32, ROWS, 128)
        out[n, :, :, rb:rb + 32, :] = o
    return out
